# revision 12
# baseline (speedup 1.0000x reference)
"""Trainium2 Bass kernel for nn_Attention_60885456388891 (gnn_message_passing).

Computation (per batch b):
  node_h = h @ W_h2node + b_h2node
  score_n[n] = sum_d tanh(p_node_feats[b,n,d] + node_h[b,d]) * w_alpha1[d]
  node_w = renorm(softmax(score_n) * att_masks)
  node_res_ = sum_n node_w[n] * node_feats[b,n,:]
  (same for relations)
  node_res = glu(cat(node_res_, rela_res_) @ W_ng + b_ng)
  rela_res = glu(cat(rela_res_, node_res) @ W_rg + b_rg)

Strategy: pure data-parallel over batch B=512 across 8 cores (64 batches/core),
all features downcast to fp16 on the host (halves HBM traffic; rel-err ~1e-3
vs the 2e-2 gate).

Per-core pipeline (v4 design):
  - pnf/prf streamed in d-on-partitions layout: broadcast-add of node_h/rela_h
    becomes a per-partition-scalar DVE add (fp16 4x-ish mode), tanh batches
    into large ACT calls.
  - scores via tiny tanh-stationary PE matmuls (FWL fast weight load):
    lhsT = tanh chunk [128d, 128n], rhs = w_alpha chunk col -> score columns
    accumulate in PSUM.  |score| <= sum|w_alpha| ~ 8, so exp() needs no max
    subtraction; masked-exp columns are used UNNORMALIZED as weights, and the
    1/sum(EM) normalizer is folded into the GLU epilogue as a per-row scale.
  - phase C: nf/rf chunk stationary [128n, 128d] x EM column -> X^T columns
    accumulate directly in the k-chunked layout the GLU matmuls consume (no
    row staging, no transposes).
  - GLU: out = (Xn_u @ Wtop) * rSn + (Xr_u @ Wbot) * rSr + bias_bcast,
    a*sigmoid(g).
"""

import numpy as np

import concourse.bass as bass
import concourse.bacc as bacc
import concourse.mybir as mybir
import concourse.tile as tile
from concourse.bass_utils import run_bass_kernel_spmd

# Problem dims (hardcoded per contract)
B, N, R, D = 512, 128, 256, 512
NCORES = 8
BS = B // NCORES          # 64 batches per core
PAIR = 4                  # batches per stream DMA block
NBLK = BS // PAIR         # 16 blocks
G = 16                    # batches per softmax/psum group
GROUPS = BS // G          # 4 groups
KC = D // 128             # 4 k-chunks of 128
KC2 = 2 * D // 128        # 8 k-chunks for the 1024-wide GLU matmuls

F32 = mybir.dt.float32
F16 = mybir.dt.float16
AF = mybir.ActivationFunctionType
ALU = mybir.AluOpType
AX = mybir.AxisListType


def build_program():
    nc = bacc.Bacc("TRN2", target_bir_lowering=False, debug=False)

    def din(name, shape, dt=F16):
        return nc.dram_tensor(name, shape, dt, kind="ExternalInput").ap()

    h_d = din("h", [BS, D])
    pnf_d = din("pnf", [NBLK, 128, PAIR, KC, N])        # d-partition args
    prf_d = din("prf", [NBLK, 128, PAIR, KC, R])
    nf_d = din("nf", [NBLK, 128, PAIR, KC, 128])        # n-partition values
    rf_d = din("rf", [NBLK, 128, PAIR, 2, KC, 128])
    mT_d = din("mT", [128, 3, BS])                      # masks, transposed
    Wn_d = din("w_h2node", [128, KC, D])
    bn_d = din("b_h2node", [1, D])
    Wr_d = din("w_h2rela", [128, KC, D])
    br_d = din("b_h2rela", [1, D])
    w1_d = din("w1c", [128, KC])                        # w_alpha1 as columns
    w2_d = din("w2c", [128, KC])
    Wng_d = din("w_ng", [128, KC2, 2, 512])
    Wrg_d = din("w_rg", [128, KC2, 2, 512])
    bng_d = din("bias_ng", [BS, 2, 512], F32)           # host-broadcast bias
    brg_d = din("bias_rg", [BS, 2, 512], F32)
    id_d = din("ident", [128, 128])                     # f16 identity
    ones_d = din("ones_col", [128, 1])                  # f16 ones column
    onesr_d = din("ones_row", [1, 128])                 # f16 ones row

    nres_d = nc.dram_tensor("node_res", [BS, D], F32, kind="ExternalOutput").ap()
    rres_d = nc.dram_tensor("rela_res", [BS, D], F32, kind="ExternalOutput").ap()

    dma = nc.sync.dma_start
    dma_s = nc.gpsimd.dma_start

    with tile.TileContext(nc) as tc:
        with (
            tc.tile_pool(name="const", bufs=1) as cp,
        ):
            # ---- persistent constants ----
            ident = cp.tile([128, 128], F16)
            dma(out=ident, in_=id_d)
            ones_col = cp.tile([128, 1], F16)
            dma(out=ones_col, in_=ones_d)
            ones_row = cp.tile([1, 128], F16)
            dma(out=ones_row, in_=onesr_d)
            w1c = cp.tile([128, KC], F16)
            dma(out=w1c, in_=w1_d)
            w2c = cp.tile([128, KC], F16)
            dma(out=w2c, in_=w2_d)
            mT = cp.tile([128, 3, BS], F16)
            dma(out=mT, in_=mT_d)

            # persistent outputs of phase C / B
            XTn = cp.tile([128, KC, BS], F16, tag="xtn")   # unnormalized Xn^T
            XTr = cp.tile([128, KC, BS], F16, tag="xtr")
            S_sb = cp.tile([1, 3, BS], F32, tag="ssb")     # EM column sums
            nhT = cp.tile([128, KC, BS], F32, tag="nht")   # bias columns
            rhT = cp.tile([128, KC, BS], F32, tag="rht")
            # GLU weights/biases (DMA'd during the last group's streaming)
            Wng_sb = cp.tile([128, KC2, 2, 512], F16, tag="wng")
            Wrg_sb = cp.tile([128, KC2, 2, 512], F16, tag="wrg")
            bng_sb = cp.tile([BS, 2, 512], F32, tag="bng")
            brg_sb = cp.tile([BS, 2, 512], F32, tag="brg")

            # ---- prologue: node_h = h @ W_h2node + b (and rela) ----
            with (
                tc.tile_pool(name="prol", bufs=1) as pp,
                tc.tile_pool(name="prps", bufs=2, space="PSUM") as pps,
            ):
                h_sb = pp.tile([BS, D], F16, tag="h")
                dma(out=h_sb, in_=h_d)
                Wn_sb = pp.tile([128, KC, D], F16, tag="wn")
                dma(out=Wn_sb, in_=Wn_d)
                Wr_sb = pp.tile([128, KC, D], F16, tag="wr")
                dma(out=Wr_sb, in_=Wr_d)
                bn_sb = pp.tile([1, D], F16, tag="bn")
                dma(out=bn_sb, in_=bn_d)
                br_sb = pp.tile([1, D], F16, tag="br")
                dma(out=br_sb, in_=br_d)

                hT = pp.tile([128, KC, BS], F16, tag="ht")
                for c in range(KC):
                    tps = pps.tile([128, BS], F16, tag="tps")
                    nc.tensor.transpose(tps, h_sb[:, c * 128:(c + 1) * 128],
                                        ident[:BS, :BS])
                    nc.vector.tensor_copy(hT[:, c, :], tps)
                for W_sb, b_sb, dstT, tg in ((Wn_sb, bn_sb, nhT, "nh"),
                                             (Wr_sb, br_sb, rhT, "rh")):
                    ps = pps.tile([BS, D], F32, tag="nhps")
                    for c in range(KC):
                        nc.tensor.matmul(ps, hT[:, c, :], W_sb[:, c, :],
                                         start=(c == 0), stop=False)
                    nc.tensor.matmul(ps, ones_row[:1, :BS], b_sb,
                                     start=False, stop=True)
                    x16 = pp.tile([BS, D], F16, tag=tg)
                    nc.vector.tensor_copy(x16, ps)
                    for c in range(KC):
                        tps = pps.tile([128, BS], F16, tag="tps")
                        nc.tensor.transpose(tps, x16[:, c * 128:(c + 1) * 128],
                                            ident[:BS, :BS])
                        nc.vector.tensor_copy(dstT[:, c, :], tps)

            # ---- main loop: per-PAIR software pipeline over 16 slots ----
            # Slot k: [pnf/prf DMA (k)] [exp/mask/S (k-1)] [phase-C mm (k-2)]
            #         [adds+tanh (k)] [score mm (k)] [nf/rf prefetch DMA (k)]
            with (
                tc.tile_pool(name="pnfp", bufs=4) as pnfp,
                tc.tile_pool(name="prfp", bufs=4) as prfp,
                tc.tile_pool(name="nfp", bufs=5) as nfp,
                tc.tile_pool(name="rfp", bufs=5) as rfp,
                tc.tile_pool(name="emp", bufs=4) as emp,
                tc.tile_pool(name="scps", bufs=3, space="PSUM") as scps,
                tc.tile_pool(name="xps", bufs=2, space="PSUM") as xps,
                tc.tile_pool(name="sps", bufs=2, space="PSUM") as sps,
            ):
                NPJ = G // PAIR       # pair slots per group
                NSLOT = NBLK          # total slots
                LAG = 2               # phase-C runs LAG slots behind phase-A
                st = {}               # per-slot live tiles

                def phase_a(k):
                    pnf = pnfp.tile([128, PAIR, KC, N], F16, tag="pnf")
                    dma(out=pnf, in_=pnf_d[k])
                    prf = prfp.tile([128, PAIR, KC, R], F16, tag="prf")
                    dma(out=prf, in_=prf_d[k])
                    st[("pnf", k)] = pnf
                    st[("prf", k)] = prf

                def phase_a_compute(k):
                    pnf = st.pop(("pnf", k))
                    prf = st.pop(("prf", k))
                    sc = scps.tile([128, 3, PAIR], F32, tag="sc")
                    st[("sc", k)] = sc
                    for i in range(PAIR):
                        b = k * PAIR + i
                        for c in range(KC):
                            nc.vector.tensor_scalar_add(
                                pnf[:, i, c, :], pnf[:, i, c, :],
                                nhT[:, c, b:b + 1])
                            nc.vector.tensor_scalar_add(
                                prf[:, i, c, :], prf[:, i, c, :],
                                rhT[:, c, b:b + 1])
                    nc.scalar.activation(pnf, pnf, AF.Tanh)
                    nc.scalar.activation(prf, prf, AF.Tanh)
                    for i in range(PAIR):
                        for c in range(KC):
                            nc.tensor.matmul(
                                sc[:, 0, i:i + 1], pnf[:, i, c, :],
                                w1c[:, c:c + 1],
                                start=(c == 0), stop=(c == KC - 1))
                        for c in range(KC):
                            nc.tensor.matmul(
                                sc[:, 1, i:i + 1], prf[:, i, c, :128],
                                w2c[:, c:c + 1],
                                start=(c == 0), stop=(c == KC - 1))
                        for c in range(KC):
                            nc.tensor.matmul(
                                sc[:, 2, i:i + 1], prf[:, i, c, 128:],
                                w2c[:, c:c + 1],
                                start=(c == 0), stop=(c == KC - 1))

                def phase_b_exp(k):
                    """masked exp for slot k (ACT + DVE, off PE's path)."""
                    sc = st.pop(("sc", k))
                    em = emp.tile([128, 3, PAIR], F16, tag="em")
                    st[("em", k)] = em
                    nc.scalar.activation(em, sc, AF.Exp)
                    b0 = k * PAIR
                    nc.vector.tensor_mul(em, em, mT[:, :, b0:b0 + PAIR])

                def phase_b_smm(k):
                    """EM column sums for slot k (PE, deps ready a slot ago)."""
                    g, j = divmod(k, NPJ)
                    em = st[("em", k)]
                    if j == 0:
                        s_tile = sps.tile([1, 3, G], F32, tag="s")
                        st[("s", g)] = s_tile
                    s_ps = st[("s", g)]
                    nc.tensor.matmul(s_ps[:, :, j * PAIR:(j + 1) * PAIR],
                                     ones_col, em, start=True, stop=True)
                    if j == NPJ - 1:
                        g0 = g * G
                        nc.vector.tensor_copy(S_sb[:, :, g0:g0 + G],
                                              st.pop(("s", g)))

                def prefetch_values(k):
                    nf = nfp.tile([128, PAIR, KC, 128], F16, tag="nf")
                    dma_s(out=nf, in_=nf_d[k])
                    rf = rfp.tile([128, PAIR, 2, KC, 128], F16, tag="rf")
                    dma_s(out=rf, in_=rf_d[k])
                    st[("nf", k)] = nf
                    st[("rf", k)] = rf

                def phase_c(k):
                    g, j = divmod(k, NPJ)
                    nf = st.pop(("nf", k))
                    rf = st.pop(("rf", k))
                    em = st.pop(("em", k))
                    if j == 0:
                        xp_tile = xps.tile([128, 2, KC, G], F32, tag="xp")
                        st[("xp", g)] = xp_tile
                    xp = st[("xp", g)]
                    for i in range(PAIR):
                        jj = j * PAIR + i
                        for c in range(KC):
                            nc.tensor.matmul(
                                xp[:, 0, c, jj:jj + 1], nf[:, i, c, :],
                                em[:, 0, i:i + 1],
                                start=True, stop=True)
                        for c in range(KC):
                            nc.tensor.matmul(
                                xp[:, 1, c, jj:jj + 1], rf[:, i, 0, c, :],
                                em[:, 1, i:i + 1],
                                start=True, stop=False)
                            nc.tensor.matmul(
                                xp[:, 1, c, jj:jj + 1], rf[:, i, 1, c, :],
                                em[:, 2, i:i + 1],
                                start=False, stop=True)
                    if j == NPJ - 1:
                        g0 = g * G
                        xp = st.pop(("xp", g))
                        nc.vector.tensor_copy(XTn[:, :, g0:g0 + G], xp[:, 0])
                        nc.vector.tensor_copy(XTr[:, :, g0:g0 + G], xp[:, 1])

                for k in range(NSLOT + LAG + 2):
                    if k < NSLOT:
                        phase_a(k)                    # pnf/prf DMA (sync queue)
                    if 2 <= k < NSLOT + 2:
                        phase_b_smm(k - 2)            # S sums (PE, deps ready)
                    if LAG + 1 <= k < NSLOT + LAG + 1:
                        phase_c(k - LAG - 1)          # weighted-sum matmuls
                    if k < NSLOT:
                        phase_a_compute(k)            # adds + tanh + score mms
                    if 1 <= k <= NSLOT:
                        phase_b_exp(k - 1)            # exp + mask
                    if k < NSLOT:
                        prefetch_values(k)            # nf/rf DMA (gpsimd queue)
                    if k == 10:
                        dma_s(out=Wng_sb, in_=Wng_d)  # GLU weight prefetch
                        dma_s(out=bng_sb, in_=bng_d)
                    elif k == 12:
                        dma_s(out=Wrg_sb, in_=Wrg_d)
                        dma_s(out=brg_sb, in_=brg_d)

            # ---- normalizers: rS columns [BS, 2] (node, rela) ----
            with (
                tc.tile_pool(name="glue", bufs=1) as gp,
                tc.tile_pool(name="glps", bufs=1, space="PSUM") as gps,
                tc.tile_pool(name="trps", bufs=2, space="PSUM") as tps_p,
            ):
                nc.vector.tensor_add(S_sb[:, 1, :], S_sb[:, 1, :], S_sb[:, 2, :])
                rS = gp.tile([1, 2, BS], F32, tag="rs")
                nc.vector.reciprocal(rS, S_sb[:, 0:2, :])
                rS16 = gp.tile([1, 2, BS], F16, tag="rs16")
                nc.vector.tensor_copy(rS16, rS)
                rSc_ps = gps.tile([BS, 2], F32, tag="rscp")
                for k in range(2):
                    nc.tensor.matmul(rSc_ps[:, k:k + 1], rS16[:, k, :],
                                     ones_col[:1, :1], start=True, stop=True)
                rSc = gp.tile([BS, 2], F32, tag="rsc")
                nc.vector.tensor_copy(rSc, rSc_ps)

                # ---- GLU heads ----
                def glu_head(lhs1, scale1_k, lhs2, scale2_k, W_sb, bias_sb,
                             out_dr, tg):
                    """out = glu((lhs1_u*rS1 | lhs2[_u*rS2]) @ W + bias)."""
                    p1 = gps.tile([BS, 2, 512], F32, tag="p1")
                    p2 = gps.tile([BS, 2, 512], F32, tag="p2")
                    for hh in range(2):
                        for c in range(KC):
                            nc.tensor.matmul(p1[:, hh, :], lhs1[:, c, :],
                                             W_sb[:, c, hh, :],
                                             start=(c == 0), stop=(c == KC - 1))
                        for c in range(KC):
                            nc.tensor.matmul(p2[:, hh, :], lhs2[:, c, :],
                                             W_sb[:, KC + c, hh, :],
                                             start=(c == 0), stop=(c == KC - 1))
                    s1 = gp.tile([BS, 2, 512], F32, tag=tg + "s1")
                    nc.vector.tensor_scalar_mul(s1, p1, rSc[:, scale1_k:scale1_k + 1])
                    if scale2_k is not None:
                        s2 = gp.tile([BS, 2, 512], F32, tag=tg + "s2")
                        nc.vector.tensor_scalar_mul(
                            s2, p2, rSc[:, scale2_k:scale2_k + 1])
                        nc.vector.tensor_add(s1, s1, s2)
                    else:
                        nc.vector.tensor_add(s1, s1, p2)
                    nc.vector.tensor_add(s1, s1, bias_sb)
                    sig = gp.tile([BS, 512], F32, tag=tg + "sig")
                    nc.scalar.activation(sig, s1[:, 1, :], AF.Sigmoid)
                    res = gp.tile([BS, 512], F32, tag=tg + "res")
                    nc.vector.tensor_mul(res, s1[:, 0, :], sig)
                    dma(out=out_dr, in_=res)
                    return res

                nres = glu_head(XTn, 0, XTr, 1, Wng_sb, bng_sb, nres_d, "ng")

                # transpose node_res for the second head
                nres16 = gp.tile([BS, D], F16, tag="n16")
                nc.vector.tensor_copy(nres16, nres)
                nresT = gp.tile([128, KC, BS], F16, tag="nrt")
                for c in range(KC):
                    tps = tps_p.tile([128, BS], F16, tag="tps2")
                    nc.tensor.transpose(tps, nres16[:, c * 128:(c + 1) * 128],
                                        ident[:BS, :BS])
                    nc.vector.tensor_copy(nresT[:, c, :], tps)

                glu_head(XTr, 1, nresT, None, Wrg_sb, brg_sb, rres_d, "rg")

    nc.compile()
    return nc


def make_in_maps(inputs):
    """Shard + lay out full inputs into 8 per-core input dicts (host-side)."""
    f16 = np.float16
    f32 = np.float32

    pnf = np.asarray(inputs["p_node_feats"], dtype=f16)
    nf = np.asarray(inputs["node_feats"], dtype=f16)
    prf = np.asarray(inputs["p_rela_feats"], dtype=f16)
    rf = np.asarray(inputs["rela_feats"], dtype=f16)
    h = np.asarray(inputs["h"], dtype=f16)
    am = np.asarray(inputs["att_masks"], dtype=f16)
    rm = np.asarray(inputs["rela_masks"], dtype=f16)

    def shuf_pnf(x):  # [BS,N,D] -> [NBLK,128,PAIR,KC,N]  (d-partition)
        x = x.reshape(NBLK, PAIR, N, KC, 128)
        return np.ascontiguousarray(x.transpose(0, 4, 1, 3, 2))

    def shuf_prf(x):  # [BS,R,D] -> [NBLK,128,PAIR,KC,R]
        x = x.reshape(NBLK, PAIR, R, KC, 128)
        return np.ascontiguousarray(x.transpose(0, 4, 1, 3, 2))

    def shuf_nf(x):  # [BS,N,D] -> [NBLK,128,PAIR,KC,128]  (n-partition)
        x = x.reshape(NBLK, PAIR, N, KC, 128)
        return np.ascontiguousarray(x.transpose(0, 2, 1, 3, 4))

    def shuf_rf(x):  # [BS,R,D] -> [NBLK,128,PAIR,2,KC,128]
        x = x.reshape(NBLK, PAIR, 2, 128, KC, 128)
        return np.ascontiguousarray(x.transpose(0, 3, 1, 2, 4, 5))

    def wcols(w):  # [D] -> [128, KC]
        return np.ascontiguousarray(
            np.asarray(w, dtype=f16).reshape(KC, 128).T)

    Wn = np.ascontiguousarray(
        np.asarray(inputs["W_h2node"], dtype=f16).reshape(KC, 128, D)
        .transpose(1, 0, 2))
    Wr = np.ascontiguousarray(
        np.asarray(inputs["W_h2rela"], dtype=f16).reshape(KC, 128, D)
        .transpose(1, 0, 2))
    Wng = np.ascontiguousarray(
        np.asarray(inputs["W_ng"], dtype=f16).reshape(KC2, 128, 2, 512)
        .transpose(1, 0, 2, 3))
    Wrg = np.ascontiguousarray(
        np.asarray(inputs["W_rg"], dtype=f16).reshape(KC2, 128, 2, 512)
        .transpose(1, 0, 2, 3))
    bng = np.ascontiguousarray(np.broadcast_to(
        np.asarray(inputs["b_ng"], dtype=f32), (BS, 2 * D)).reshape(BS, 2, 512))
    brg = np.ascontiguousarray(np.broadcast_to(
        np.asarray(inputs["b_rg"], dtype=f32), (BS, 2 * D)).reshape(BS, 2, 512))

    shared = {
        "w_h2node": Wn, "w_h2rela": Wr,
        "b_h2node": np.asarray(inputs["b_h2node"], dtype=f16).reshape(1, D),
        "b_h2rela": np.asarray(inputs["b_h2rela"], dtype=f16).reshape(1, D),
        "w1c": wcols(inputs["w_alpha1"]),
        "w2c": wcols(inputs["w_alpha2"]),
        "w_ng": Wng, "w_rg": Wrg, "bias_ng": bng, "bias_rg": brg,
        "ident": np.eye(128, dtype=f16),
        "ones_col": np.ones((128, 1), dtype=f16),
        "ones_row": np.ones((1, 128), dtype=f16),
    }
    in_maps = []
    for cix in range(NCORES):
        s = slice(cix * BS, (cix + 1) * BS)
        mT = np.empty((128, 3, BS), dtype=f16)
        mT[:, 0, :] = am[s].T
        mT[:, 1, :] = rm[s, :128].T
        mT[:, 2, :] = rm[s, 128:].T
        in_maps.append({
            "h": np.ascontiguousarray(h[s]),
            "pnf": shuf_pnf(pnf[s]), "prf": shuf_prf(prf[s]),
            "nf": shuf_nf(nf[s]), "rf": shuf_rf(rf[s]),
            "mT": np.ascontiguousarray(mT), **shared,
        })
    return in_maps


_NC_CACHE = None
LAST_RESULTS = None  # BassKernelResults of the most recent kernel() call


def kernel(**inputs):
    global _NC_CACHE, LAST_RESULTS
    if _NC_CACHE is None:
        _NC_CACHE = build_program()
    nc = _NC_CACHE
    in_maps = make_in_maps(inputs)
    import os
    trace = os.environ.get("BASS_KERNEL_TRACE", "0") == "1"
    res = run_bass_kernel_spmd(nc, in_maps, core_ids=list(range(NCORES)),
                               trace=trace)
    LAST_RESULTS = res
    node_res = np.concatenate([r["node_res"] for r in res.results], axis=0)
    rela_res = np.concatenate([r["rela_res"] for r in res.results], axis=0)
    return node_res, rela_res
